# revision 5
# baseline (speedup 1.0000x reference)
"""Multi-head self-attention with RoPE on 8 Trainium2 NeuronCores.

Full inputs in, full output out. Sharding: batch (2) x head-groups (4 heads
per core). Each core computes qkv projections for its heads, RoPE, full
softmax(QK^T)V, and a partial output projection; host sums the 4 partials
per batch element and adds b_out.

Problem shape: B=2, T=2048, D=1024, H=16, HD=64 (hardcoded).
"""

import numpy as np
from contextlib import ExitStack

import concourse.bass as bass
import concourse.mybir as mybir
import concourse.tile as tile
from concourse import bass_utils

B, T, D, H = 2, 2048, 1024, 16
HD = 64          # head dim
HL = 4           # heads per core
N_CORES = 8
ROPE_BASE = 10000.0

F32 = mybir.dt.float32
F32R = mybir.dt.float32r

Exp = mybir.ActivationFunctionType.Exp

# results of the last run (for test harness introspection)
LAST_RESULTS = None
TRACE = False


def _split_excess_waits(nc, cap=1):
    """walrus in this env rejects >1 sync-wait per instruction; split extras
    onto single-wait NoOps on the same engine queue."""
    n = 0
    for f in nc.m.functions:
        for bb in f.blocks:
            insts = bb.instructions
            if not any(
                i.sync_info is not None and len(i.sync_info.on_wait) > cap
                for i in insts
            ):
                continue
            out = []
            for inst in insts:
                si = inst.sync_info
                waits = list(si.on_wait) if si is not None else []
                if len(waits) > cap:
                    extra, keep = waits[:-cap], waits[-cap:]
                    for k, w in enumerate(extra):
                        nop = mybir.InstNoOp(
                            name=f"{inst.name}-ws{k}",
                            engine=inst.engine,
                            sync_info=mybir.SyncInfo(on_wait=[w], on_update=[]),
                            bass_nofuse=True,
                        )
                        nc.register_instruction(nop)
                        out.append(nop)
                        n += 1
                    inst.sync_info = mybir.SyncInfo(
                        on_wait=keep, on_update=list(si.on_update)
                    )
                out.append(inst)
            bb.instructions = out
    return n


def _build_bass():
    nc = bass.Bass("TRN2", target_bir_lowering=False, debug=False, num_devices=1)

    # ---- DRAM I/O ----
    d_xT = nc.dram_tensor("xT", [D, T], F32R, kind="ExternalInput").ap()
    d_wqk = nc.dram_tensor("wqk", [D, 2 * HL * HD], F32R, kind="ExternalInput").ap()
    d_wv = nc.dram_tensor("wv", [D, HL * (HD + 1)], F32R, kind="ExternalInput").ap()
    d_bqk = nc.dram_tensor("bqk", [1, 2 * HL * HD], F32R, kind="ExternalInput").ap()
    d_bv = nc.dram_tensor("bv", [1, HL * (HD + 1)], F32R, kind="ExternalInput").ap()
    d_ones = nc.dram_tensor("ones", [1, T], F32R, kind="ExternalInput").ap()
    d_cos = nc.dram_tensor("cos2", [128, T], F32, kind="ExternalInput").ap()
    d_sin = nc.dram_tensor("sin2", [128, T], F32, kind="ExternalInput").ap()
    d_rT = nc.dram_tensor("rT", [128, 128], F32R, kind="ExternalInput").ap()
    d_ind = nc.dram_tensor("ind", [HL, 2 * 128], F32R, kind="ExternalInput").ap()
    d_amask = nc.dram_tensor("amask", [128, T // 128], F32, kind="ExternalInput").ap()
    d_wo = nc.dram_tensor("wo", [HL * HD, D], F32R, kind="ExternalInput").ap()
    d_out = nc.dram_tensor("out_part", [T, D], F32, kind="ExternalOutput").ap()

    NT = T // 128            # 16 token tiles
    NK = D // 128            # 8 contraction chunks
    SC = HD ** -0.5          # softmax scale

    with tile.TileContext(nc) as tc, ExitStack() as ctx:
        pool = lambda st, name, bufs: st.enter_context(tc.tile_pool(name=name, bufs=bufs))
        psum = lambda st, name, bufs: st.enter_context(
            tc.tile_pool(name=name, bufs=bufs, space="PSUM")
        )

        # lifetime-grouped pools: g_load dies after phase 2, g_att2/g_fin open late
        g_load = ctx.enter_context(ExitStack())
        psA = ctx.enter_context(ExitStack())

        p_const = pool(ctx, "const", 1)
        p_qkT = pool(ctx, "qkT", 4)
        p_v = pool(ctx, "v", NT)
        p_xt = pool(g_load, "xt", NK)
        p_w = pool(g_load, "w", NK)
        p_wv = pool(g_load, "wv", NK)
        p_cs = pool(g_load, "cossin", 1)
        p_tmp = pool(g_load, "tmp", 2)

        ps_qk = psum(psA, "ps_qk", 2)
        ps_rot = psum(psA, "ps_rot", 2)

        # ---- constants / tables ----
        t_ones = p_const.tile([1, 512], F32R, tag="ones")
        nc.sync.dma_start(t_ones[:], d_ones[:, 0:512])
        t_bqk = p_const.tile([1, 2 * HL * HD], F32R, tag="bqk")
        nc.sync.dma_start(t_bqk[:], d_bqk[:])
        t_bv = p_const.tile([1, HL * (HD + 1)], F32R, tag="bv")
        nc.sync.dma_start(t_bv[:], d_bv[:])
        t_cos = p_cs.tile([128, T], F32, tag="cos")
        nc.sync.dma_start(t_cos[:], d_cos[:])
        t_sin = p_cs.tile([128, T], F32, tag="sin")
        nc.sync.dma_start(t_sin[:], d_sin[:])
        t_rT = p_const.tile([128, 128], F32R, tag="rT")
        nc.sync.dma_start(t_rT[:], d_rT[:])
        t_ind = p_const.tile([HL, 2 * 128], F32R, tag="ind")
        nc.sync.dma_start(t_ind[:], d_ind[:])
        t_amask = p_const.tile([128, T // 128], F32, tag="amask")
        nc.sync.dma_start(t_amask[:], d_amask[:])

        # ---- weight / input loads ----
        xt = []
        for k in range(NK):
            tk = p_xt.tile([128, T], F32R, tag="xt")
            nc.sync.dma_start(tk[:], d_xT[k * 128:(k + 1) * 128, :])
            xt.append(tk)
        wqk = []
        for k in range(NK):
            tk = p_w.tile([128, 2 * HL * HD], F32R, tag="wqk")
            nc.sync.dma_start(tk[:], d_wqk[k * 128:(k + 1) * 128, :])
            wqk.append(tk)
        wv = []
        for k in range(NK):
            tk = p_wv.tile([128, HL * (HD + 1)], F32R, tag="wv")
            nc.sync.dma_start(tk[:], d_wv[k * 128:(k + 1) * 128, :])
            wv.append(tk)

        # ---- phase 1: q/k projections (feature-major) + RoPE ----
        # chunks 0,1 = qT heads (0,1),(2,3); chunks 2,3 = kT likewise
        qkT = []
        for c2 in range(4):
            t_qk = p_qkT.tile([128, T], F32R, tag="qkT")
            qkT.append(t_qk)
            for ih in range(2):  # halves of the token axis
                sl = slice(ih * (T // 2), (ih + 1) * (T // 2))
                pqk = ps_qk.tile([128, T // 2], F32, tag="pqk")
                for k in range(NK):
                    for n5 in range(2):
                        s5 = slice(n5 * 512, (n5 + 1) * 512)
                        g5 = slice(ih * (T // 2) + n5 * 512,
                                   ih * (T // 2) + (n5 + 1) * 512)
                        nc.tensor.matmul(
                            pqk[:, s5],
                            wqk[k][:, c2 * 128:(c2 + 1) * 128],
                            xt[k][:, g5],
                            start=(k == 0),
                            stop=False,
                            skip_group_check=True,
                        )
                # bias (b_qkv slice) via K=1 matmul: adds bqk[f] to every token
                for n5 in range(2):
                    s5 = slice(n5 * 512, (n5 + 1) * 512)
                    g5 = slice(ih * (T // 2) + n5 * 512,
                               ih * (T // 2) + (n5 + 1) * 512)
                    nc.tensor.matmul(
                        pqk[:, s5],
                        t_bqk[:, c2 * 128:(c2 + 1) * 128],
                        t_ones[:, 0:512],
                        start=False,
                        stop=True,
                        skip_group_check=True,
                    )
                # RoPE: roped = raw*cos + R @ (raw*sin)   (sin is 32-symmetric)
                u_sb = p_tmp.tile([128, T // 2], F32R, tag="u")
                nc.vector.tensor_mul(u_sb[:], pqk[:], t_sin[:, sl])
                prot = ps_rot.tile([128, T // 2], F32, tag="prot")
                for n5 in range(2):
                    s5 = slice(n5 * 512, (n5 + 1) * 512)
                    nc.tensor.matmul(
                        prot[:, s5], t_rT[:], u_sb[:, s5],
                        start=True, stop=True, skip_group_check=True,
                    )
                c_sb = p_tmp.tile([128, T // 2], F32, tag="c")
                nc.vector.tensor_mul(c_sb[:], pqk[:], t_cos[:, sl])
                nc.vector.tensor_add(t_qk[:, sl], c_sb[:], prot[:])

        # ---- phase 2: v projection (token-major, interleaved + ones col) ----
        psA.close()
        psB = ctx.enter_context(ExitStack())
        ps_v = psum(psB, "ps_v", 2)
        VW = HL * (HD + 1)  # 260
        v_sb = []
        for t in range(NT):
            pv_ps = ps_v.tile([128, VW], F32, tag="pv_ps")
            for k in range(NK):
                nc.tensor.matmul(
                    pv_ps[:],
                    xt[k][:, t * 128:(t + 1) * 128],
                    wv[k][:],
                    start=(k == 0),
                    stop=False,
                    skip_group_check=True,
                )
            # bias + ones column (bv has 1.0 at the ones slots)
            nc.tensor.matmul(
                pv_ps[:], t_ones[:, 0:128], t_bv[:],
                start=False, stop=True, skip_group_check=True,
            )
            vt = p_v.tile([128, VW], F32R, tag="v")
            nc.vector.tensor_copy(vt[:], pv_ps[:])
            v_sb.append(vt)

        # ---- phase 3: attention per head ----
        g_load.close()
        psB.close()
        psC = ctx.enter_context(ExitStack())
        ps_s = psum(psC, "ps_s", 2)
        ps_pv = psum(psC, "ps_pv", 1)
        p_e = pool(ctx, "eT", 3)
        p_a = pool(ctx, "aT", HL)
        p_fin = ctx.enter_context(ExitStack())
        p_anorm = pool(p_fin, "anorm", 2)
        p_wo = pool(p_fin, "wo", 2)
        p_osb = pool(p_fin, "osb", 2)
        p_small = pool(p_fin, "small", 1)
        a_sb = []
        for h in range(HL):
            c2 = h // 2
            base = (h % 2) * 64
            qh = qkT[c2]
            kh = qkT[2 + c2]
            pv = ps_pv.tile([HD + 1, T], F32, tag="pv")
            for jb in range(NT):
                for ih in range(2):
                    sps = ps_s.tile([128, T // 2], F32, tag="sT")
                    for n5 in range(2):
                        s5 = slice(n5 * 512, (n5 + 1) * 512)
                        g5 = slice(ih * (T // 2) + n5 * 512,
                                   ih * (T // 2) + (n5 + 1) * 512)
                        nc.tensor.matmul(
                            sps[:, s5],
                            kh[base:base + HD, jb * 128:(jb + 1) * 128],
                            qh[base:base + HD, g5],
                            start=True, stop=True, skip_group_check=True,
                        )
                    e_sb = p_e.tile([128, T // 2], F32R, tag="eT")
                    nc.scalar.activation(
                        e_sb[:], sps[:], Exp,
                        bias=t_amask[:, jb:jb + 1], scale=SC,
                    )
                    for n5 in range(2):
                        s5 = slice(n5 * 512, (n5 + 1) * 512)
                        g5 = slice(ih * (T // 2) + n5 * 512,
                                   ih * (T // 2) + (n5 + 1) * 512)
                        nc.tensor.matmul(
                            pv[:, g5],
                            v_sb[jb][:, h * (HD + 1):(h + 1) * (HD + 1)],
                            e_sb[:, s5],
                            start=(jb == 0),
                            stop=(jb == NT - 1),
                            skip_group_check=True,
                        )
            at = p_a.tile([HD + 1, T], F32, tag="aT")
            nc.vector.tensor_copy(at[:], pv[:])
            a_sb.append(at)

        # ---- phase 4: softmax normalization ----
        psC.close()
        psD = ctx.enter_context(ExitStack())
        ps_b = psum(psD, "ps_b", 1)
        # gather the 4 sums rows into [128, 64] (p-major: t = p*16 + c)
        sums128 = p_small.tile([128, HL * (T // 128)], F32, tag="sums128")
        for h in range(HL):
            nc.sync.dma_start(
                sums128[:, h * (T // 128):(h + 1) * (T // 128)],
                a_sb[h][HD:HD + 1, :].rearrange("o (p c) -> o p c", p=128),
            )
        recip128 = p_small.tile([128, HL * (T // 128)], F32, tag="recip128")
        nc.vector.reciprocal(recip128[:], sums128[:])
        recip_all = p_small.tile([HL, T], F32R, tag="recip_all")
        for h in range(HL):
            nc.sync.dma_start(
                recip_all[h:h + 1, :].rearrange("o (p c) -> o p c", p=128),
                recip128[:, h * (T // 128):(h + 1) * (T // 128)].bitcast(F32R),
            )
        # broadcast recip rows to [128, T] per chunk and multiply
        anorm = []
        for c2 in range(2):
            # move the two heads' aT rows into one [128, T] tile
            ar = p_anorm.tile([128, T], F32, tag="anorm_raw")
            nc.sync.dma_start(ar[0:HD, :], a_sb[2 * c2][0:HD, :])
            nc.sync.dma_start(ar[HD:2 * HD, :], a_sb[2 * c2 + 1][0:HD, :])
            pb = ps_b.tile([128, T], F32, tag="pb")
            for n5 in range(T // 512):
                s5 = slice(n5 * 512, (n5 + 1) * 512)
                nc.tensor.matmul(
                    pb[:, s5],
                    t_ind[:, c2 * 128:(c2 + 1) * 128],
                    recip_all[:, s5],
                    start=True, stop=True, skip_group_check=True,
                )
            an = p_anorm.tile([128, T], F32R, tag="anorm")
            nc.vector.tensor_mul(an[:], pb[:], ar[:])
            anorm.append(an)

        # ---- phase 5: output projection (partial over local heads) ----
        psD.close()
        psE = ctx.enter_context(ExitStack())
        ps_p = psum(psE, "ps_p", 2)
        wo_sb = []
        for c2 in range(2):
            wt = p_wo.tile([128, D], F32R, tag="wo")
            nc.sync.dma_start(wt[:], d_wo[c2 * 128:(c2 + 1) * 128, :])
            wo_sb.append(wt)
        for t in range(NT):
            pp = ps_p.tile([128, D], F32, tag="pp")
            for c2 in range(2):
                for n5 in range(2):
                    s5 = slice(n5 * 512, (n5 + 1) * 512)
                    nc.tensor.matmul(
                        pp[:, s5],
                        anorm[c2][:, t * 128:(t + 1) * 128],
                        wo_sb[c2][:, s5],
                        start=(c2 == 0),
                        stop=(c2 == 1),
                        skip_group_check=True,
                    )
            osb = p_osb.tile([128, D], F32, tag="osb")
            nc.vector.tensor_copy(osb[:], pp[:])
            nc.sync.dma_start(d_out[t * 128:(t + 1) * 128, :], osb[:])

    _split_excess_waits(nc)
    return nc


_NC_CACHE = None


def _rope_tables():
    inv_freq = (1.0 / (ROPE_BASE ** (np.arange(0, HD, 2, dtype=np.float32) / HD))
                ).astype(np.float32)
    t = np.arange(T, dtype=np.float32)
    freqs = np.einsum("t,f->tf", t, inv_freq).astype(np.float32)  # (T, HD/2)
    emb = np.concatenate([freqs, freqs], axis=-1)                  # (T, HD)
    cosT = np.cos(emb).astype(np.float32).T                        # (HD, T)
    sinT = np.sin(emb).astype(np.float32).T
    cos2 = np.ascontiguousarray(np.tile(cosT, (2, 1)))             # (128, T)
    sin2 = np.ascontiguousarray(np.tile(sinT, (2, 1)))
    return cos2, sin2


def _rot_matrix():
    r = np.zeros((128, 128), dtype=np.float32)
    for p0 in (0, 64):
        for d in range(32):
            r[p0 + d, p0 + 32 + d] = -1.0
            r[p0 + 32 + d, p0 + d] = 1.0
    return np.ascontiguousarray(r.T)


def kernel(x, W_qkv, b_qkv, W_out, b_out, padding_mask):
    global _NC_CACHE, LAST_RESULTS
    x = np.asarray(x, dtype=np.float32)
    W_qkv = np.asarray(W_qkv, dtype=np.float32)
    b_qkv = np.asarray(b_qkv, dtype=np.float32)
    W_out = np.asarray(W_out, dtype=np.float32)
    b_out = np.asarray(b_out, dtype=np.float32)
    padding_mask = np.asarray(padding_mask)

    if _NC_CACHE is None:
        _NC_CACHE = _build_bass()
    nc = _NC_CACHE

    cos2, sin2 = _rope_tables()
    rT = _rot_matrix()

    ind = np.zeros((HL, 2 * 128), dtype=np.float32)
    for c2 in range(2):
        for f in range(128):
            ind[2 * c2 + f // 64, c2 * 128 + f] = 1.0

    ones = np.ones((1, T), dtype=np.float32)

    in_maps = []
    for c in range(N_CORES):
        b = c // 4
        g = c % 4
        q0 = g * HL * HD
        wq = W_qkv[:, q0:q0 + HL * HD]
        wk = W_qkv[:, D + q0:D + q0 + HL * HD]
        wv_flat = W_qkv[:, 2 * D + q0:2 * D + q0 + HL * HD]
        # interleave v columns with a zero (ones-slot) column per head
        wv_aug = np.zeros((D, HL * (HD + 1)), dtype=np.float32)
        bv_aug = np.zeros((1, HL * (HD + 1)), dtype=np.float32)
        for h in range(HL):
            wv_aug[:, h * (HD + 1):h * (HD + 1) + HD] = wv_flat[:, h * HD:(h + 1) * HD]
            bv_aug[0, h * (HD + 1):h * (HD + 1) + HD] = \
                b_qkv[2 * D + q0 + h * HD:2 * D + q0 + (h + 1) * HD]
            bv_aug[0, h * (HD + 1) + HD] = 1.0
        bqk = np.concatenate(
            [b_qkv[q0:q0 + HL * HD], b_qkv[D + q0:D + q0 + HL * HD]]
        ).reshape(1, -1).astype(np.float32)
        amask = np.where(padding_mask[b], np.float32(-1e30), np.float32(0.0))
        amask = np.ascontiguousarray(amask.reshape(T // 128, 128).T.astype(np.float32))
        in_maps.append({
            "xT": np.ascontiguousarray(x[b].T),
            "wqk": np.ascontiguousarray(np.concatenate([wq, wk], axis=1)),
            "wv": wv_aug,
            "bqk": bqk,
            "bv": bv_aug,
            "ones": ones,
            "cos2": cos2,
            "sin2": sin2,
            "rT": rT,
            "ind": ind,
            "amask": amask,
            "wo": np.ascontiguousarray(W_out[q0:q0 + HL * HD, :]),
        })

    res = bass_utils.run_bass_kernel_spmd(
        nc, in_maps, core_ids=list(range(N_CORES)), trace=TRACE,
    )
    LAST_RESULTS = res

    out = np.zeros((B, T, D), dtype=np.float64)
    for c in range(N_CORES):
        out[c // 4] += res.results[c]["out_part"].astype(np.float64)
    out += b_out.astype(np.float64)
    return out.astype(np.float32)
